# revision 5
# baseline (speedup 1.0000x reference)
"""Trainium2 Bass kernel for nn_ConditionalMMDecoder (GRU decoder with two
Bahdanau attentions + large vocab log-softmax loss).

Strategy (8 NeuronCores, SPMD, no collectives):
  - Batch-parallel: each core owns B'=8 of the 64 batch rows for the whole
    recurrence and computes the full-V sum(exp(z)) for its own 47*8=376 rows.
  - Hoisted out of the 47-step scan: ctx->bottleneck attention projections,
    the fusion pre-projection of ctx (P = ctx @ Wf_half.T, so the softmax
    weighted sum directly produces the fusion input), and gi0 = emb(y) @ W.T.
  - Transposed GRU state layout [128h, 4hc, 8b] so gate elementwise work runs
    on full 128-partition tiles; GRU weights are bf16 stationary operands.
  - loss = sum_t,b (logsumexp_V(z) - z[target]); the V-sized pass computes
    row sum(exp(z)) on device (f32 psum, ACT exp+accum).  z[target] is also
    computed on device (elementwise L*Wv[:,target] + reduce) so the kernel's
    only output is one tiny [128,4] f32 tensor per core: per-row-tile sumexp
    in cols 0..2 and the core's sum_m z_target scalar at [0,3].  (A large
    ExternalOutput tensor costs ~78ms/call of runtime overhead in this
    dispatch path - measured - so all bulk results stay on device.)
  - All device inputs are packed host-side into one [128, NPK] bf16 tensor
    (plus a small [128,*] f32 one and the [E,V] vocab matrix), so every SBUF
    load is a contiguous per-partition DMA.
"""
import sys
import os

sys.path.insert(0, "/opt/trn_rl_repo")

import numpy as np
import ml_dtypes

import concourse.bass as bass
import concourse.tile as tile
import concourse.mybir as mybir
from concourse.bass_utils import run_bass_kernel_spmd
from concourse.masks import make_identity

bf16 = mybir.dt.bfloat16
f32 = mybir.dt.float32
AF = mybir.ActivationFunctionType
OP = mybir.AluOpType

# problem constants (hardcoded per contract)
H = 512
E = 512
C = 512
V = 32000
B = 64
T = 48
S_T = 50
S_I = 196
NCORES = 8
BP = B // NCORES            # batch per core = 8
NSTEPS = T - 1              # 47
M = NSTEPS * BP             # 376 logit rows per core
MPAD = 384                  # padded to 3 m-tiles of 128
NEG = -1e9
VS = 512                    # vocab slice
NVS = (V + VS - 1) // VS    # 63 slices (62 full + 256)

# bf16 pack layout: (name, elems-per-partition). Order shared between
# build_nc and the host packer.
PACK_BASE = [
    ("w0h", 4 * 3 * H // 1),   # r4(W0hT)      [128, 4, 1536]
    ("w1i", 4 * 3 * H),
    ("w1h", 4 * 3 * H),
    ("hdt", 4 * C),
    ("hdi", 4 * C),
    ("h2o", 4 * E),
    ("vtt", 4),
    ("vit", 4),
    ("ind", 2 * 128),          # [8, 2, 128] on partitions 0..7
    ("wtg", 4 * M),            # r4(WvT[:, y_next]) [128, 4, 376]
    ("ctb", 4 * BP * S_T),
    ("cib", 4 * BP * S_I),
    ("yeb", 4 * NSTEPS * BP),
    ("c2t", 4 * C),
    ("c2i", 4 * C),
    ("wft", 4 * H),
    ("wfi", 4 * H),
    ("w0i", 4 * 3 * H),
]

_cache = {}
LAST_NC = None
LAST_IN_MAPS = None


def _pack_layout(use_bhh0, use_b1i, use_b1h):
    items = list(PACK_BASE)
    if use_bhh0:
        items.append(("a0h", 3 * H))   # [1, 1536] on partition 0
    if use_b1i:
        items.append(("a1i", 3 * H))
    if use_b1h:
        items.append(("a1h", 3 * H))
    off = {}
    o = 0
    for n, sz in items:
        off[n] = o
        o += sz
    return off, o


def _packf_layout(use_mask):
    items = [("h2ob", 4), ("b0f", 12)]
    if use_mask:
        items.append(("madd", BP))
    off = {}
    o = 0
    for n, sz in items:
        off[n] = o
        o += sz
    return off, o


def _split_waits(nc):
    """walrus in this container accepts only one sem wait per instruction;
    move extra waits onto preceding nops on the same engine."""
    ctr = 0
    for func in nc.m.functions:
        for bb in func.blocks:
            newlist = []
            for ins in bb.instructions:
                si = ins.sync_info
                waits = list(si.on_wait) if (si and si.on_wait) else []
                if len(waits) > 1:
                    for wchunk in waits[:-1]:
                        ctr += 1
                        newlist.append(mybir.InstNoOp(
                            name=f"waitfix-{ctr}",
                            engine=ins.engine,
                            sync_info=mybir.SyncInfo(on_wait=[wchunk], on_update=[]),
                        ))
                    si.on_wait = waits[-1:]
                newlist.append(ins)
            bb.instructions[:] = newlist
    return nc


def build_nc(nsteps=NSTEPS, use_mask=False, use_bhh0=False, use_b1i=False,
             use_b1h=False, use_bv=False, phase=4):
    nc = bass.Bass()

    poff, NPK = _pack_layout(use_bhh0, use_b1i, use_b1h)
    foff, NPKF = _packf_layout(use_mask)

    pk = nc.dram_tensor("pk", [128, NPK], bf16, kind="ExternalInput")
    pkf = nc.dram_tensor("pkf", [128, NPKF], f32, kind="ExternalInput")
    WvT = nc.dram_tensor("WvT", [E, V], bf16, kind="ExternalInput")
    if use_bv:
        bvT = nc.dram_tensor("bvT", [1, V], bf16, kind="ExternalInput")

    S_out = nc.dram_tensor("S", [128, 4], f32, kind="ExternalOutput")

    def ld(name, k):
        return pk[:, poff[name]:poff[name] + dict(PACK_BASE)[name]] \
            .rearrange("p (k d) -> p k d", k=k)

    with tile.TileContext(nc) as tc:
        with tc.tile_pool(name="singles", bufs=1) as sg:
            # ---- persistent sbuf tiles + input DMAs (contiguous pack slices)
            w0h = sg.tile([128, 4, 3 * H], bf16)
            nc.sync.dma_start(out=w0h, in_=ld("w0h", 4))
            w1i = sg.tile([128, 4, 3 * H], bf16)
            nc.sync.dma_start(out=w1i, in_=ld("w1i", 4))
            w1h = sg.tile([128, 4, 3 * H], bf16)
            nc.sync.dma_start(out=w1h, in_=ld("w1h", 4))
            hdt = sg.tile([128, 4, C], bf16)
            nc.sync.dma_start(out=hdt, in_=ld("hdt", 4))
            hdi = sg.tile([128, 4, C], bf16)
            nc.sync.dma_start(out=hdi, in_=ld("hdi", 4))
            h2o = sg.tile([128, 4, E], bf16)
            nc.sync.dma_start(out=h2o, in_=ld("h2o", 4))
            vtt = sg.tile([128, 4, 1], bf16)
            nc.sync.dma_start(out=vtt, in_=ld("vtt", 4))
            vit = sg.tile([128, 4, 1], bf16)
            nc.sync.dma_start(out=vit, in_=ld("vit", 4))
            ind = sg.tile([8, 2, 128], bf16)
            nc.sync.dma_start(
                out=ind,
                in_=pk[0:8, poff["ind"]:poff["ind"] + 256]
                .rearrange("b (g q) -> b g q", g=2))
            wtg = sg.tile([128, 4, M], bf16)
            nc.sync.dma_start(out=wtg, in_=ld("wtg", 4))
            h2obt = sg.tile([128, 4], f32)
            nc.sync.dma_start(out=h2obt, in_=pkf[:, foff["h2ob"]:foff["h2ob"] + 4])
            b0f = sg.tile([128, 12], f32)
            nc.sync.dma_start(out=b0f, in_=pkf[:, foff["b0f"]:foff["b0f"] + 12])
            if use_mask:
                maddt = sg.tile([128, BP], f32)
                nc.sync.dma_start(out=maddt,
                                  in_=pkf[:, foff["madd"]:foff["madd"] + BP])
            if use_bhh0:
                a0h = sg.tile([1, 3 * H], bf16)
                nc.sync.dma_start(out=a0h, in_=pk[0:1, poff["a0h"]:poff["a0h"] + 3 * H])
            if use_b1i:
                a1i = sg.tile([1, 3 * H], bf16)
                nc.sync.dma_start(out=a1i, in_=pk[0:1, poff["a1i"]:poff["a1i"] + 3 * H])
            if use_b1h:
                a1h = sg.tile([1, 3 * H], bf16)
                nc.sync.dma_start(out=a1h, in_=pk[0:1, poff["a1h"]:poff["a1h"] + 3 * H])
            if use_bv:
                bvt = sg.tile([1, V], bf16)
                nc.sync.dma_start(out=bvt, in_=bvT[:, :])

            ident = sg.tile([128, 128], f32)
            make_identity(nc, ident[:, :])
            ones_bf = sg.tile([128, 8], bf16)
            nc.vector.memset(ones_bf[:, :], 1.0)
            ones_row = sg.tile([1, 128], bf16)
            nc.vector.memset(ones_row[:, :], 1.0)
            ones_f = sg.tile([128, 1], f32)
            nc.vector.memset(ones_f[:, :], 1.0)

            # persistent work tiles
            gi0 = sg.tile([128, 12, nsteps, BP], f32)
            hist = sg.tile([128, 4, nsteps + 1, BP], f32)
            histbf = sg.tile([128, 4, nsteps + 1, BP], bf16)
            nc.vector.memset(hist[:, :, :, :], 0.0)
            nc.vector.memset(histbf[:, :, :, :], 0.0)
            pjt = sg.tile([128, 4, BP, S_T], bf16)
            pji = sg.tile([128, 4, BP, S_I], bf16)
            Pt = sg.tile([128, BP, H], bf16)
            Pi0 = sg.tile([128, BP, H], bf16)
            Pi1 = sg.tile([128, BP, H], bf16)
            LTb = sg.tile([128, 4, MPAD], bf16)
            nc.vector.memset(LTb[:, :, :], 0.0)

            # =============== hoist phase ===============
            with tc.tile_pool(name="hoist_sb", bufs=1) as hsb, \
                 tc.tile_pool(name="hoist_ps", bufs=2, space="PSUM") as hps:
                ctb = hsb.tile([128, 4, BP, S_T], bf16)
                nc.sync.dma_start(
                    out=ctb,
                    in_=pk[:, poff["ctb"]:poff["ctb"] + 4 * BP * S_T]
                    .rearrange("p (k b s) -> p k b s", k=4, b=BP))
                cib = hsb.tile([128, 4, BP, S_I], bf16)
                nc.sync.dma_start(
                    out=cib,
                    in_=pk[:, poff["cib"]:poff["cib"] + 4 * BP * S_I]
                    .rearrange("p (k b s) -> p k b s", k=4, b=BP))
                yeb = hsb.tile([128, 4, nsteps * BP], bf16)
                nc.sync.dma_start(
                    out=yeb,
                    in_=pk[:, poff["yeb"]:poff["yeb"] + 4 * nsteps * BP]
                    .rearrange("p (k m) -> p k m", k=4))
                c2t = hsb.tile([128, 4, C], bf16)
                nc.sync.dma_start(out=c2t, in_=ld("c2t", 4))
                c2i = hsb.tile([128, 4, C], bf16)
                nc.sync.dma_start(out=c2i, in_=ld("c2i", 4))
                wft = hsb.tile([128, 4, H], bf16)
                nc.sync.dma_start(out=wft, in_=ld("wft", 4))
                wfi = hsb.tile([128, 4, H], bf16)
                nc.sync.dma_start(out=wfi, in_=ld("wfi", 4))
                w0i = hsb.tile([128, 4, 3 * H], bf16)
                nc.sync.dma_start(out=w0i, in_=ld("w0i", 4))

                # gi0 = yemb @ W0i.T + b0fold
                for mt in range(12):
                    p = hps.tile([128, nsteps * BP], f32, tag="gi0")
                    for kc in range(4):
                        nc.tensor.matmul(p[:, :], w0i[:, kc, mt * 128:(mt + 1) * 128],
                                         yeb[:, kc, :], start=(kc == 0), stop=(kc == 3))
                    nc.vector.tensor_scalar(
                        out=gi0[:, mt, :, :].rearrange("p t b -> p (t b)"),
                        in0=p[:, :],
                        scalar1=b0f[:, mt:mt + 1], scalar2=None, op0=OP.add)

                # projT txt: [128d, 400] per d-chunk
                for dk in range(4):
                    p = hps.tile([128, BP, S_T], f32, tag="pjt")
                    for kc in range(4):
                        nc.tensor.matmul(p[:, :, :], c2t[:, kc, dk * 128:(dk + 1) * 128],
                                         ctb[:, kc, :, :], start=(kc == 0), stop=(kc == 3))
                    nc.vector.tensor_copy(pjt[:, dk, :, :], p[:, :, :])
                # projT img: rhs split in b-pairs (N=392), one bank per pair
                for dk in range(4):
                    for q in range(4):
                        p = hps.tile([128, 2, S_I], f32, tag="pji")
                        for kc in range(4):
                            nc.tensor.matmul(p[:, :, :], c2i[:, kc, dk * 128:(dk + 1) * 128],
                                             cib[:, kc, 2 * q:2 * q + 2, :],
                                             start=(kc == 0), stop=(kc == 3))
                        nc.vector.tensor_copy(pji[:, dk, 2 * q:2 * q + 2, :], p[:, :, :])

                # P tiles (fusion pre-projection of ctx)
                for b in range(BP):
                    p = hps.tile([128, H], f32, tag="P")
                    for kc in range(4):
                        nc.tensor.matmul(p[:S_T, :], ctb[:, kc, b, :], wft[:, kc, :],
                                         start=(kc == 0), stop=(kc == 3))
                    nc.vector.tensor_copy(Pt[0:S_T, b, :], p[0:S_T, :])
                for b in range(BP):
                    p = hps.tile([128, H], f32, tag="P")
                    for kc in range(4):
                        nc.tensor.matmul(p[:, :], cib[:, kc, b, 0:128], wfi[:, kc, :],
                                         start=(kc == 0), stop=(kc == 3))
                    nc.vector.tensor_copy(Pi0[:, b, :], p[:, :])
                    p = hps.tile([128, H], f32, tag="P")
                    for kc in range(4):
                        nc.tensor.matmul(p[:S_I - 128, :], cib[:, kc, b, 128:S_I], wfi[:, kc, :],
                                         start=(kc == 0), stop=(kc == 3))
                    nc.vector.tensor_copy(Pi1[0:S_I - 128, b, :], p[0:S_I - 128, :])

            # =============== recurrence ===============
            with tc.tile_pool(name="ps_gate", bufs=1, space="PSUM") as psg, \
                 tc.tile_pool(name="ps_g1n", bufs=1, space="PSUM") as psn, \
                 tc.tile_pool(name="ps_ht", bufs=1, space="PSUM") as psh, \
                 tc.tile_pool(name="ps_sc", bufs=1, space="PSUM") as pssc, \
                 tc.tile_pool(name="ps_den", bufs=1, space="PSUM") as psd, \
                 tc.tile_pool(name="ps_fz", bufs=1, space="PSUM") as psf, \
                 tc.tile_pool(name="ps_tr", bufs=1, space="PSUM") as pst, \
                 tc.tile_pool(name="step", bufs=2) as st, \
                 tc.tile_pool(name="epool", bufs=1) as ep:

                for t in range(nsteps if phase >= 2 else 0):
                    # ---- GRU0: gh0.T = W0h stationary @ h.T
                    pg = psg.tile([128, 12, BP], f32, tag="g")
                    for kc in range(4):
                        for mt in range(12):
                            nc.tensor.matmul(pg[:, mt, :], w0h[:, kc, mt * 128:(mt + 1) * 128],
                                             histbf[:, kc, t, :],
                                             start=(kc == 0),
                                             stop=(kc == 3 and not use_bhh0))
                    if use_bhh0:
                        for mt in range(12):
                            nc.tensor.matmul(pg[:, mt, :], a0h[:, mt * 128:(mt + 1) * 128],
                                             ones_bf[0:1, 0:BP], start=False, stop=True)
                    Arz = st.tile([128, 8, BP], f32, tag="Arz")
                    nc.vector.tensor_tensor(
                        out=Arz[:, :, :], in0=gi0[:, 0:8, t, :],
                        in1=pg[:, 0:8, :], op=OP.add)
                    r0 = st.tile([128, 4, BP], f32, tag="r0")
                    # sigmoid(x) = 0.5 + 0.5*tanh(x/2): keeps the ACT engine on
                    # one table (Tanh) all step - saves ~3 table loads/step
                    nc.scalar.activation(r0[:, :, :], Arz[:, 0:4, :], AF.Tanh, scale=0.5)
                    nc.vector.tensor_scalar(out=r0[:, :, :], in0=r0[:, :, :],
                                            scalar1=0.5, scalar2=0.5,
                                            op0=OP.mult, op1=OP.add)
                    z0 = st.tile([128, 4, BP], f32, tag="z0")
                    nc.scalar.activation(z0[:, :, :], Arz[:, 4:8, :], AF.Tanh, scale=0.5)
                    nc.vector.tensor_scalar(out=z0[:, :, :], in0=z0[:, :, :],
                                            scalar1=0.5, scalar2=0.5,
                                            op0=OP.mult, op1=OP.add)
                    tn = st.tile([128, 4, BP], f32, tag="tn")
                    nc.vector.tensor_tensor(out=tn[:, :, :], in0=r0[:, :, :],
                                            in1=pg[:, 8:12, :], op=OP.mult)
                    nc.vector.tensor_tensor(out=tn[:, :, :], in0=tn[:, :, :],
                                            in1=gi0[:, 8:12, t, :], op=OP.add)
                    n0 = st.tile([128, 4, BP], f32, tag="n0")
                    nc.scalar.activation(n0[:, :, :], tn[:, :, :], AF.Tanh)
                    # h1 = n0 + z0*(h - n0)
                    d0 = st.tile([128, 4, BP], f32, tag="d0")
                    nc.vector.tensor_tensor(out=d0[:, :, :], in0=hist[:, :, t, :],
                                            in1=n0[:, :, :], op=OP.subtract)
                    nc.vector.tensor_tensor(out=d0[:, :, :], in0=z0[:, :, :],
                                            in1=d0[:, :, :], op=OP.mult)
                    h1 = st.tile([128, 4, BP], f32, tag="h1")
                    nc.vector.tensor_tensor(out=h1[:, :, :], in0=n0[:, :, :],
                                            in1=d0[:, :, :], op=OP.add)
                    h1b = st.tile([128, 4, BP], bf16, tag="h1b")
                    nc.vector.tensor_copy(h1b[:, :, :], h1[:, :, :])

                    # ---- hterm.T = hid2ctx stationary @ h1.T  (both modalities)
                    ph = psh.tile([128, 2, 4, BP], f32, tag="ht")
                    for kc in range(4):
                        for dk in range(4):
                            nc.tensor.matmul(ph[:, 0, dk, :], hdt[:, kc, dk * 128:(dk + 1) * 128],
                                             h1b[:, kc, :], start=(kc == 0), stop=(kc == 3))
                            nc.tensor.matmul(ph[:, 1, dk, :], hdi[:, kc, dk * 128:(dk + 1) * 128],
                                             h1b[:, kc, :], start=(kc == 0), stop=(kc == 3))
                    htT = st.tile([128, 2, 4, BP], f32, tag="htT")
                    nc.vector.tensor_copy(
                        htT[:, :, :, :].rearrange("p m k b -> p (m k b)"),
                        ph[:, :, :, :].rearrange("p m k b -> p (m k b)"))

                    # ---- e = tanh(proj + hterm) via ACT bias, per (dchunk, b)
                    eTt = ep.tile([128, 4, BP, S_T], bf16, tag="eTt")
                    eTi = ep.tile([128, 4, BP, S_I], bf16, tag="eTi")
                    for dk in range(4):
                        for b in range(BP):
                            nc.scalar.activation(eTt[:, dk, b, :], pjt[:, dk, b, :],
                                                 AF.Tanh, bias=htT[:, 0, dk, b:b + 1])
                            nc.scalar.activation(eTi[:, dk, b, :], pji[:, dk, b, :],
                                                 AF.Tanh, bias=htT[:, 1, dk, b:b + 1])

                    # ---- scores.T [s, b] = e.T stationary @ v
                    psc = pssc.tile([128, 3, BP], f32, tag="sc")
                    for b in range(BP):
                        for dk in range(4):
                            nc.tensor.matmul(psc[0:S_T, 0, b:b + 1], eTt[:, dk, b, :],
                                             vtt[:, dk, :], start=(dk == 0), stop=(dk == 3))
                            nc.tensor.matmul(psc[0:128, 1, b:b + 1], eTi[:, dk, b, 0:128],
                                             vit[:, dk, :], start=(dk == 0), stop=(dk == 3))
                            nc.tensor.matmul(psc[0:S_I - 128, 2, b:b + 1], eTi[:, dk, b, 128:S_I],
                                             vit[:, dk, :], start=(dk == 0), stop=(dk == 3))
                    if use_mask:
                        nc.vector.tensor_tensor(out=psc[0:S_T, 0, :], in0=psc[0:S_T, 0, :],
                                                in1=maddt[0:S_T, :], op=OP.add)
                    # ---- w = exp(scores)
                    wTt = st.tile([128, BP], bf16, tag="wTt")
                    wTi0 = st.tile([128, BP], bf16, tag="wTi0")
                    wTi1 = st.tile([128, BP], bf16, tag="wTi1")
                    nc.scalar.activation(wTt[0:S_T, :], psc[0:S_T, 0, :], AF.Exp)
                    nc.scalar.activation(wTi0[:, :], psc[:, 1, :], AF.Exp)
                    nc.scalar.activation(wTi1[0:S_I - 128, :], psc[0:S_I - 128, 2, :], AF.Exp)

                    # ---- denominators + reciprocal scatter
                    pd = psd.tile([128, 8], f32, tag="den")
                    nc.tensor.matmul(pd[0:8, 0:1], wTt[0:S_T, :], ones_bf[0:S_T, 0:1],
                                     start=True, stop=True)
                    nc.tensor.matmul(pd[0:8, 1:2], wTi0[:, :], ones_bf[:, 0:1],
                                     start=True, stop=False)
                    nc.tensor.matmul(pd[0:8, 1:2], wTi1[0:S_I - 128, :], ones_bf[0:S_I - 128, 0:1],
                                     start=False, stop=True)
                    rdf = st.tile([8, 2], f32, tag="rdf")
                    nc.vector.reciprocal(rdf[:, :], pd[0:8, 0:2])
                    rdb = st.tile([8, 2], bf16, tag="rdb")
                    nc.vector.tensor_copy(rdb[:, :], rdf[:, :])
                    for g in range(2):
                        nc.tensor.matmul(pd[:, 2 + 2 * g:4 + 2 * g], ind[:, g, :], rdb[:, :],
                                         start=True, stop=True)
                    rds = st.tile([128, 2, 2], f32, tag="rds")
                    nc.vector.tensor_copy(rds[:, :, :].rearrange("p g x -> p (g x)"),
                                          pd[:, 2:6])

                    # ---- weighted sums of P (fusion input), col-packed 4 b/bank
                    fzpre = st.tile([128, 2, H], f32, tag="fzpre")
                    tmpc = st.tile([128, H], f32, tag="tmpc")
                    for g in range(2):
                        pa = psf.tile([128, H], f32, tag="fzA")
                        pb = psf.tile([128, H], f32, tag="fzB")
                        for j in range(4):
                            b = 4 * g + j
                            nc.tensor.matmul(pa[32 * j:32 * j + 1, :], wTt[0:S_T, b:b + 1],
                                             Pt[0:S_T, b, :], start=True, stop=True,
                                             tile_position=(0, 32 * j))
                            nc.tensor.matmul(pb[32 * j:32 * j + 1, :], wTi0[:, b:b + 1],
                                             Pi0[:, b, :], start=True, stop=False,
                                             tile_position=(0, 32 * j))
                            nc.tensor.matmul(pb[32 * j:32 * j + 1, :], wTi1[0:S_I - 128, b:b + 1],
                                             Pi1[0:S_I - 128, b, :], start=False, stop=True,
                                             tile_position=(0, 32 * j))
                        nc.vector.tensor_scalar(out=tmpc[:, :], in0=pb[:, :],
                                                scalar1=rds[:, g, 1:2], scalar2=None,
                                                op0=OP.mult)
                        nc.vector.scalar_tensor_tensor(
                            out=fzpre[:, g, :], in0=pa[:, :], scalar=rds[:, g, 0:1],
                            in1=tmpc[:, :], op0=OP.mult, op1=OP.add)
                    fzf = st.tile([128, 2, H], f32, tag="fzf")
                    nc.scalar.activation(fzf[:, :, :], fzpre[:, :, :], AF.Tanh)

                    # ---- transpose fz [8b, 512] -> fzT [128c, 4kc, 8b]
                    fzT = st.tile([128, 4, BP], bf16, tag="fzT")
                    for g in range(2):
                        for ck in range(4):
                            ptr = pst.tile([128, 128], f32, tag="tr")
                            nc.tensor.transpose(ptr[:, :], fzf[:, g, ck * 128:(ck + 1) * 128],
                                                ident[:, :])
                            nc.vector.tensor_copy(fzT[:, ck, 4 * g:4 * g + 4],
                                                  ptr[:, 0:128:32])

                    # ---- GRU1
                    pg1 = psg.tile([128, 12, BP], f32, tag="g")
                    pn1 = psn.tile([128, 4, BP], f32, tag="gn")
                    for kc in range(4):
                        for mt in range(12):
                            nc.tensor.matmul(pg1[:, mt, :], w1i[:, kc, mt * 128:(mt + 1) * 128],
                                             fzT[:, kc, :], start=(kc == 0),
                                             stop=(kc == 3 and mt >= 8 and not use_b1i))
                    if use_b1i:
                        for mt in range(12):
                            nc.tensor.matmul(pg1[:, mt, :], a1i[:, mt * 128:(mt + 1) * 128],
                                             ones_bf[0:1, 0:BP], start=False, stop=(mt >= 8))
                    for kc in range(4):
                        for mt in range(8):
                            nc.tensor.matmul(pg1[:, mt, :], w1h[:, kc, mt * 128:(mt + 1) * 128],
                                             h1b[:, kc, :], start=False,
                                             stop=(kc == 3 and not use_b1h))
                        for mt in range(4):
                            nc.tensor.matmul(pn1[:, mt, :], w1h[:, kc, (8 + mt) * 128:(9 + mt) * 128],
                                             h1b[:, kc, :], start=(kc == 0),
                                             stop=(kc == 3 and not use_b1h))
                    if use_b1h:
                        for mt in range(8):
                            nc.tensor.matmul(pg1[:, mt, :], a1h[:, mt * 128:(mt + 1) * 128],
                                             ones_bf[0:1, 0:BP], start=False, stop=True)
                        for mt in range(4):
                            nc.tensor.matmul(pn1[:, mt, :], a1h[:, (8 + mt) * 128:(9 + mt) * 128],
                                             ones_bf[0:1, 0:BP], start=False, stop=True)
                    r1 = st.tile([128, 4, BP], f32, tag="r0")
                    nc.scalar.activation(r1[:, :, :], pg1[:, 0:4, :], AF.Tanh, scale=0.5)
                    nc.vector.tensor_scalar(out=r1[:, :, :], in0=r1[:, :, :],
                                            scalar1=0.5, scalar2=0.5,
                                            op0=OP.mult, op1=OP.add)
                    z1 = st.tile([128, 4, BP], f32, tag="z0")
                    nc.scalar.activation(z1[:, :, :], pg1[:, 4:8, :], AF.Tanh, scale=0.5)
                    nc.vector.tensor_scalar(out=z1[:, :, :], in0=z1[:, :, :],
                                            scalar1=0.5, scalar2=0.5,
                                            op0=OP.mult, op1=OP.add)
                    tn1 = st.tile([128, 4, BP], f32, tag="tn")
                    nc.vector.tensor_tensor(out=tn1[:, :, :], in0=r1[:, :, :],
                                            in1=pn1[:, :, :], op=OP.mult)
                    nc.vector.tensor_tensor(out=tn1[:, :, :], in0=tn1[:, :, :],
                                            in1=pg1[:, 8:12, :], op=OP.add)
                    n1 = st.tile([128, 4, BP], f32, tag="n0")
                    nc.scalar.activation(n1[:, :, :], tn1[:, :, :], AF.Tanh)
                    d1 = st.tile([128, 4, BP], f32, tag="d0")
                    nc.vector.tensor_tensor(out=d1[:, :, :], in0=h1[:, :, :],
                                            in1=n1[:, :, :], op=OP.subtract)
                    nc.vector.tensor_tensor(out=d1[:, :, :], in0=z1[:, :, :],
                                            in1=d1[:, :, :], op=OP.mult)
                    nc.vector.tensor_tensor(out=hist[:, :, t + 1, :], in0=n1[:, :, :],
                                            in1=d1[:, :, :], op=OP.add)
                    nc.vector.tensor_copy(histbf[:, :, t + 1, :], hist[:, :, t + 1, :])

            # =============== logits + vocab phase ===============
            m_rows = nsteps * BP
            nmt = (m_rows + 127) // 128
            with tc.tile_pool(name="ps_L", bufs=2, space="PSUM") as psL:
              if phase >= 3:
                  for e in range(4):
                      p = psL.tile([128, m_rows], f32, tag="L")
                      for kc in range(4):
                          nc.tensor.matmul(
                              p[:, :], h2o[:, kc, e * 128:(e + 1) * 128],
                              histbf[:, kc, 1:nsteps + 1, :].rearrange("p t b -> p (t b)"),
                              start=(kc == 0), stop=(kc == 3))
                      nc.scalar.activation(LTb[:, e, 0:m_rows], p[:, :], AF.Tanh,
                                           bias=h2obt[:, e:e + 1])

            with tc.tile_pool(name="wv", bufs=3) as wvp, \
                 tc.tile_pool(name="ps_z", bufs=4, space="PSUM") as psz, \
                 tc.tile_pool(name="ps_zt", bufs=1, space="PSUM") as pszt, \
                 tc.tile_pool(name="vocab_sb", bufs=3) as vsb:
              if phase >= 4:
                  Sacc = sg.tile([128, 3, NVS], f32)
                  Srow = sg.tile([128, 4], f32)
                  for vs in range(NVS):
                      n = min(VS, V - vs * VS)
                      wvt = wvp.tile([128, 4, VS], bf16, tag="wv")
                      nc.sync.dma_start(
                          out=wvt[:, :, 0:n],
                          in_=WvT[:, vs * VS:vs * VS + n].rearrange("(k p) v -> p k v", p=128))
                      for mt in range(nmt):
                          pz = psz.tile([128, VS], f32, tag="z")
                          for e in range(4):
                              nc.tensor.matmul(pz[:, 0:n], LTb[:, e, mt * 128:(mt + 1) * 128],
                                               wvt[:, e, 0:n], start=(e == 0),
                                               stop=(e == 3 and not use_bv))
                          if use_bv:
                              nc.tensor.matmul(pz[:, 0:n], ones_row[0:1, :],
                                               bvt[:, vs * VS:vs * VS + n], start=False, stop=True)
                          scr = vsb.tile([128, VS], bf16, tag="scr")
                          nc.scalar.activation(scr[:, 0:n], pz[:, 0:n], AF.Exp,
                                               accum_out=Sacc[:, mt, vs:vs + 1])
                  for mt in range(nmt):
                      nc.vector.reduce_sum(Srow[:, mt:mt + 1], Sacc[:, mt, :],
                                           axis=mybir.AxisListType.X)

                  # ---- z_target on device: ztot = sum_m L[m,:].Wv[:,y_next[m]]
                  ztmp = sg.tile([128, 4, M], f32)
                  nc.vector.tensor_tensor(out=ztmp[:, :, :], in0=LTb[:, :, 0:M],
                                          in1=wtg[:, :, :], op=OP.mult)
                  ztp = sg.tile([128, 1], f32)
                  nc.vector.reduce_sum(ztp[:, 0:1],
                                       ztmp[:, :, :].rearrange("p k m -> p (k m)"),
                                       axis=mybir.AxisListType.X)
                  pzt = pszt.tile([128, 1], f32, tag="zt")
                  nc.tensor.matmul(pzt[0:1, 0:1], ztp[:, 0:1], ones_f[:, 0:1],
                                   start=True, stop=True)
                  nc.vector.tensor_copy(Srow[0:1, 3:4], pzt[0:1, 0:1])
                  nc.sync.dma_start(out=S_out[:, :], in_=Srow[:, :])

    _split_waits(nc)
    return nc


def _to_bf(x):
    return np.asarray(x, dtype=np.float32).astype(ml_dtypes.bfloat16)


def _r4flat(a):
    """[512, D] -> [128, 4*D] with block[p, k*D+d] = a[k*128+p, d]."""
    a = np.asarray(a)
    D = a.shape[1]
    return np.ascontiguousarray(a.reshape(4, 128, D).transpose(1, 0, 2).reshape(128, 4 * D))


def kernel(**inputs):
    txt_ctx = np.asarray(inputs["txt_ctx"], np.float32)
    txt_mask = np.asarray(inputs["txt_mask"], np.float32)
    img_ctx = np.asarray(inputs["img_ctx"], np.float32)
    y = np.asarray(inputs["y"])
    emb_W = np.asarray(inputs["emb_W"], np.float32)
    d0Wih = np.asarray(inputs["dec0_Wih"], np.float32)
    d0Whh = np.asarray(inputs["dec0_Whh"], np.float32)
    d0bih = np.asarray(inputs["dec0_bih"], np.float32)
    d0bhh = np.asarray(inputs["dec0_bhh"], np.float32)
    d1Wih = np.asarray(inputs["dec1_Wih"], np.float32)
    d1Whh = np.asarray(inputs["dec1_Whh"], np.float32)
    d1bih = np.asarray(inputs["dec1_bih"], np.float32)
    d1bhh = np.asarray(inputs["dec1_bhh"], np.float32)
    t_c2c = np.asarray(inputs["txt_ctx2ctx"], np.float32)
    t_h2c = np.asarray(inputs["txt_hid2ctx"], np.float32)
    t_v = np.asarray(inputs["txt_mlp_v"], np.float32)
    i_c2c = np.asarray(inputs["img_ctx2ctx"], np.float32)
    i_h2c = np.asarray(inputs["img_hid2ctx"], np.float32)
    i_v = np.asarray(inputs["img_mlp_v"], np.float32)
    fusion_W = np.asarray(inputs["fusion_W"], np.float32)
    h2oW = np.asarray(inputs["hid2out_W"], np.float32)
    h2ob_v = np.asarray(inputs["hid2out_b"], np.float32)
    o2pW = np.asarray(inputs["out2prob_W"], np.float32)
    o2pb = np.asarray(inputs["out2prob_b"], np.float32)

    use_mask = not np.all(txt_mask > 0)
    use_bhh0 = bool(np.any(d0bhh != 0))
    use_b1i = bool(np.any(d1bih != 0))
    use_b1h = bool(np.any(d1bhh != 0))
    use_bv = bool(np.any(o2pb != 0))

    nsteps = NSTEPS
    m_rows = nsteps * BP
    key = ("nc", nsteps, use_mask, use_bhh0, use_b1i, use_b1h, use_bv)
    if key not in _cache:
        _cache[key] = build_nc(nsteps, use_mask, use_bhh0, use_b1i, use_b1h, use_bv)
    nc = _cache[key]

    poff, NPK = _pack_layout(use_bhh0, use_b1i, use_b1h)
    foff, NPKF = _packf_layout(use_mask)

    embz = emb_W.copy()
    embz[0, :] = 0.0

    WvT_b = _to_bf(o2pW.T)                       # [E, V] bf16

    # shared (batch-independent) bf16 pack blocks
    pk_shared = np.zeros((128, NPK), ml_dtypes.bfloat16)

    def put(name, block):
        sz = dict(PACK_BASE).get(name)
        if sz is None:
            sz = 3 * H
        assert block.shape == (128, sz) or block.shape[1] == sz, (name, block.shape)
        pk_shared[:, poff[name]:poff[name] + block.shape[1]] = block

    put("w0h", _r4flat(_to_bf(d0Whh.T)))
    put("w1i", _r4flat(_to_bf(d1Wih.T)))
    put("w1h", _r4flat(_to_bf(d1Whh.T)))
    put("hdt", _r4flat(_to_bf(t_h2c.T)))
    put("hdi", _r4flat(_to_bf(i_h2c.T)))
    put("h2o", _r4flat(_to_bf(h2oW.T)))
    put("vtt", _r4flat(_to_bf(t_v[:, None])))
    put("vit", _r4flat(_to_bf(i_v[:, None])))
    put("c2t", _r4flat(_to_bf(t_c2c.T)))
    put("c2i", _r4flat(_to_bf(i_c2c.T)))
    put("wft", _r4flat(_to_bf(fusion_W[:, 0:C].T)))
    put("wfi", _r4flat(_to_bf(fusion_W[:, C:2 * C].T)))
    put("w0i", _r4flat(_to_bf(d0Wih.T)))
    ind = np.zeros((2, 8, 128), np.float32)
    for b in range(8):
        ind[b // 4, b, 32 * (b % 4)] = 1.0
    indblk = np.zeros((128, 256), ml_dtypes.bfloat16)
    indblk[0:8, :] = _to_bf(ind).transpose(1, 0, 2).reshape(8, 256)
    put("ind", indblk)
    if use_bhh0:
        blk = np.zeros((128, 3 * H), ml_dtypes.bfloat16)
        blk[0, :] = _to_bf(d0bhh)
        pk_shared[:, poff["a0h"]:poff["a0h"] + 3 * H] = blk
    if use_b1i:
        blk = np.zeros((128, 3 * H), ml_dtypes.bfloat16)
        blk[0, :] = _to_bf(d1bih)
        pk_shared[:, poff["a1i"]:poff["a1i"] + 3 * H] = blk
    if use_b1h:
        blk = np.zeros((128, 3 * H), ml_dtypes.bfloat16)
        blk[0, :] = _to_bf(d1bhh)
        pk_shared[:, poff["a1h"]:poff["a1h"] + 3 * H] = blk

    # shared f32 pack
    b0fold_v = d0bih.copy()
    b0fold_v[0:2 * H] += d0bhh[0:2 * H]
    pkf_shared = np.zeros((128, NPKF), np.float32)
    pkf_shared[:, foff["h2ob"]:foff["h2ob"] + 4] = h2ob_v.reshape(4, 128).T
    pkf_shared[:, foff["b0f"]:foff["b0f"] + 12] = b0fold_v.reshape(12, 128).T

    in_maps = []
    for c in range(NCORES):
        bs = slice(c * BP, (c + 1) * BP)
        y_c = y[:, bs].astype(np.int64)
        yemb = embz[y_c[0:nsteps].reshape(-1)].reshape(nsteps, BP, E)
        y_next = y_c[1:nsteps + 1].reshape(-1)          # [M] targets, m = t*BP+b
        pk = pk_shared.copy()
        # ctxT blocks: [512(c), BP, S] -> r4flat over (b,s)
        ct = _to_bf(txt_ctx[:, bs, :].transpose(2, 1, 0))      # [C, BP, S_T]
        pk[:, poff["ctb"]:poff["ctb"] + 4 * BP * S_T] = \
            _r4flat(ct.reshape(C, BP * S_T))
        ci = _to_bf(img_ctx[:, bs, :].transpose(2, 1, 0))      # [C, BP, S_I]
        pk[:, poff["cib"]:poff["cib"] + 4 * BP * S_I] = \
            _r4flat(ci.reshape(C, BP * S_I))
        ye = _to_bf(yemb.transpose(2, 0, 1))                   # [E, nsteps, BP]
        pk[:, poff["yeb"]:poff["yeb"] + 4 * nsteps * BP] = \
            _r4flat(ye.reshape(E, nsteps * BP))
        # wtg: WvT columns at targets, but m-index must match LTb's m = t*BP+b
        wt = WvT_b[:, y_next]                                  # [E, M] bf16
        pk[:, poff["wtg"]:poff["wtg"] + 4 * M] = _r4flat(wt)
        m = {"pk": pk, "pkf": pkf_shared, "WvT": WvT_b}
        if use_mask:
            pf = pkf_shared.copy()
            madd = np.zeros((128, BP), np.float32)
            madd[0:S_T, :] = np.where(txt_mask[:, bs] > 0, 0.0, NEG)
            pf[:, foff["madd"]:foff["madd"] + BP] = madd
            m["pkf"] = pf
        if use_bv:
            m["bvT"] = _to_bf(o2pb[None, :])
        in_maps.append(m)

    global LAST_NC, LAST_IN_MAPS
    LAST_NC, LAST_IN_MAPS = nc, in_maps
    res = run_bass_kernel_spmd(nc, in_maps, core_ids=list(range(NCORES)))

    # host reduction: loss = sum log(S_row) - sum z_target
    total = np.float64(0.0)
    for c in range(NCORES):
        r = res.results[c]
        S_flat = r["S"][:, 0:3].T.reshape(-1)[:m_rows]     # row-major m = mt*128+p
        ztot = np.float64(r["S"][0, 3])
        total += np.log(S_flat.astype(np.float64)).sum() - ztot
        if use_bv:
            # device ztot covers L@Wv[:,y]; the out2prob bias at the target
            # (included in the device logits for S) is added here
            y_next_c = y[1:nsteps + 1, c * BP:(c + 1) * BP].astype(np.int64).reshape(-1)
            total -= np.float64(o2pb[y_next_c].astype(np.float64).sum())
    return np.float32(total)


if __name__ == "__main__":
    pass


# revision 7
# speedup vs baseline: 1.0773x; 1.0773x over previous
"""Trainium2 Bass kernel for nn_ConditionalMMDecoder (GRU decoder with two
Bahdanau attentions + large vocab log-softmax loss).

Strategy (8 NeuronCores, SPMD, no collectives):
  - Batch-parallel: each core owns B'=8 of the 64 batch rows for the whole
    recurrence and computes the full-V sum(exp(z)) for its own 47*8=376 rows.
  - Hoisted out of the 47-step scan: ctx->bottleneck attention projections,
    the fusion pre-projection of ctx (P = ctx @ Wf_half.T, so the softmax
    weighted sum directly produces the fusion input), and gi0 = emb(y) @ W.T.
  - Transposed GRU state layout [128h, 4hc, 8b] so gate elementwise work runs
    on full 128-partition tiles; GRU weights are bf16 stationary operands.
  - loss = sum_t,b (logsumexp_V(z) - z[target]); the V-sized pass computes
    row sum(exp(z)) on device (f32 psum, ACT exp+accum).  z[target] is also
    computed on device (elementwise L*Wv[:,target] + reduce) so the kernel's
    only output is one tiny [128,4] f32 tensor per core: per-row-tile sumexp
    in cols 0..2 and the core's sum_m z_target scalar at [0,3].  (A large
    ExternalOutput tensor costs ~78ms/call of runtime overhead in this
    dispatch path - measured - so all bulk results stay on device.)
  - All device inputs are packed host-side into one [128, NPK] bf16 tensor
    (plus a small [128,*] f32 one and the [E,V] vocab matrix), so every SBUF
    load is a contiguous per-partition DMA.
"""
import sys
import os

sys.path.insert(0, "/opt/trn_rl_repo")

import numpy as np
import ml_dtypes

import concourse.bass as bass
import concourse.tile as tile
import concourse.mybir as mybir
from concourse.bass_utils import run_bass_kernel_spmd
from concourse.masks import make_identity

bf16 = mybir.dt.bfloat16
f32 = mybir.dt.float32
AF = mybir.ActivationFunctionType
OP = mybir.AluOpType

# problem constants (hardcoded per contract)
H = 512
E = 512
C = 512
V = 32000
B = 64
T = 48
S_T = 50
S_I = 196
NCORES = 8
BP = B // NCORES            # batch per core = 8
NSTEPS = T - 1              # 47
M = NSTEPS * BP             # 376 logit rows per core
MPAD = 384                  # padded to 3 m-tiles of 128
NEG = -1e9
VS = 512                    # vocab slice
NVS = (V + VS - 1) // VS    # 63 slices (62 full + 256)

# bf16 pack layout: (name, elems-per-partition). Order shared between
# build_nc and the host packer.
PACK_BASE = [
    ("w0h", 4 * 3 * H // 1),   # r4(W0hT)      [128, 4, 1536]
    ("w1i", 4 * 3 * H),
    ("w1h", 4 * 3 * H),
    ("hdt", 4 * C),
    ("hdi", 4 * C),
    ("h2o", 4 * E),
    ("vtt", 4),
    ("vit", 4),
    ("ind", 2 * 128),          # [8, 2, 128] on partitions 0..7
    ("wtg", 4 * M),            # r4(WvT[:, y_next]) [128, 4, 376]
    ("ctb", 4 * BP * S_T),
    ("cib", 4 * BP * S_I),
    ("yeb", 4 * NSTEPS * BP),
    ("c2t", 4 * C),
    ("c2i", 4 * C),
    ("wft", 4 * H),
    ("wfi", 4 * H),
    ("w0i", 4 * 3 * H),
]

_cache = {}
LAST_NC = None
LAST_IN_MAPS = None


def _pack_layout(use_bhh0, use_b1i, use_b1h):
    items = list(PACK_BASE)
    if use_bhh0:
        items.append(("a0h", 3 * H))   # [1, 1536] on partition 0
    if use_b1i:
        items.append(("a1i", 3 * H))
    if use_b1h:
        items.append(("a1h", 3 * H))
    off = {}
    o = 0
    for n, sz in items:
        off[n] = o
        o += sz
    return off, o


def _packf_layout(use_mask):
    items = [("h2ob", 4), ("b0f", 12)]
    if use_mask:
        items.append(("madd", BP))
    off = {}
    o = 0
    for n, sz in items:
        off[n] = o
        o += sz
    return off, o


def _split_waits(nc):
    """walrus in this container accepts only one sem wait per instruction;
    move extra waits onto preceding nops on the same engine."""
    ctr = 0
    for func in nc.m.functions:
        for bb in func.blocks:
            newlist = []
            for ins in bb.instructions:
                si = ins.sync_info
                waits = list(si.on_wait) if (si and si.on_wait) else []
                if len(waits) > 1:
                    for wchunk in waits[:-1]:
                        ctr += 1
                        newlist.append(mybir.InstNoOp(
                            name=f"waitfix-{ctr}",
                            engine=ins.engine,
                            sync_info=mybir.SyncInfo(on_wait=[wchunk], on_update=[]),
                        ))
                    si.on_wait = waits[-1:]
                newlist.append(ins)
            bb.instructions[:] = newlist
    return nc


def build_nc(nsteps=NSTEPS, use_mask=False, use_bhh0=False, use_b1i=False,
             use_b1h=False, use_bv=False, phase=4):
    nc = bass.Bass()

    poff, NPK = _pack_layout(use_bhh0, use_b1i, use_b1h)
    foff, NPKF = _packf_layout(use_mask)

    pk = nc.dram_tensor("pk", [128, NPK], bf16, kind="ExternalInput")
    pkf = nc.dram_tensor("pkf", [128, NPKF], f32, kind="ExternalInput")
    WvT = nc.dram_tensor("WvT", [E, V], bf16, kind="ExternalInput")
    if use_bv:
        bvT = nc.dram_tensor("bvT", [1, V], bf16, kind="ExternalInput")

    S_out = nc.dram_tensor("S", [128, 4], f32, kind="ExternalOutput")

    def ld(name, k):
        return pk[:, poff[name]:poff[name] + dict(PACK_BASE)[name]] \
            .rearrange("p (k d) -> p k d", k=k)

    with tile.TileContext(nc) as tc:
        with tc.tile_pool(name="singles", bufs=1) as sg:
            # ---- persistent sbuf tiles + input DMAs (contiguous pack slices)
            w0h = sg.tile([128, 4, 3 * H], bf16)
            nc.sync.dma_start(out=w0h, in_=ld("w0h", 4))
            w1i = sg.tile([128, 4, 3 * H], bf16)
            nc.sync.dma_start(out=w1i, in_=ld("w1i", 4))
            w1h = sg.tile([128, 4, 3 * H], bf16)
            nc.sync.dma_start(out=w1h, in_=ld("w1h", 4))
            hdt = sg.tile([128, 4, C], bf16)
            nc.sync.dma_start(out=hdt, in_=ld("hdt", 4))
            hdi = sg.tile([128, 4, C], bf16)
            nc.sync.dma_start(out=hdi, in_=ld("hdi", 4))
            h2o = sg.tile([128, 4, E], bf16)
            nc.sync.dma_start(out=h2o, in_=ld("h2o", 4))
            vtt = sg.tile([128, 4, 1], bf16)
            nc.sync.dma_start(out=vtt, in_=ld("vtt", 4))
            vit = sg.tile([128, 4, 1], bf16)
            nc.sync.dma_start(out=vit, in_=ld("vit", 4))
            ind = sg.tile([8, 2, 128], bf16)
            nc.sync.dma_start(
                out=ind,
                in_=pk[0:8, poff["ind"]:poff["ind"] + 256]
                .rearrange("b (g q) -> b g q", g=2))
            wtg = sg.tile([128, 4, M], bf16)
            nc.sync.dma_start(out=wtg, in_=ld("wtg", 4))
            h2obt = sg.tile([128, 4], f32)
            nc.sync.dma_start(out=h2obt, in_=pkf[:, foff["h2ob"]:foff["h2ob"] + 4])
            b0f = sg.tile([128, 12], f32)
            nc.sync.dma_start(out=b0f, in_=pkf[:, foff["b0f"]:foff["b0f"] + 12])
            if use_mask:
                maddt = sg.tile([128, BP], f32)
                nc.sync.dma_start(out=maddt,
                                  in_=pkf[:, foff["madd"]:foff["madd"] + BP])
            if use_bhh0:
                a0h = sg.tile([1, 3 * H], bf16)
                nc.sync.dma_start(out=a0h, in_=pk[0:1, poff["a0h"]:poff["a0h"] + 3 * H])
            if use_b1i:
                a1i = sg.tile([1, 3 * H], bf16)
                nc.sync.dma_start(out=a1i, in_=pk[0:1, poff["a1i"]:poff["a1i"] + 3 * H])
            if use_b1h:
                a1h = sg.tile([1, 3 * H], bf16)
                nc.sync.dma_start(out=a1h, in_=pk[0:1, poff["a1h"]:poff["a1h"] + 3 * H])
            if use_bv:
                bvt = sg.tile([1, V], bf16)
                nc.sync.dma_start(out=bvt, in_=bvT[:, :])

            ident = sg.tile([128, 128], f32)
            make_identity(nc, ident[:, :])
            ones_bf = sg.tile([128, 8], bf16)
            nc.vector.memset(ones_bf[:, :], 1.0)
            ones_row = sg.tile([1, 128], bf16)
            nc.vector.memset(ones_row[:, :], 1.0)
            ones_f = sg.tile([128, 1], f32)
            nc.vector.memset(ones_f[:, :], 1.0)

            # persistent work tiles
            gi0 = sg.tile([128, 12, nsteps, BP], f32)
            hist = sg.tile([128, 4, nsteps + 1, BP], f32)
            histbf = sg.tile([128, 4, nsteps + 1, BP], bf16)
            nc.vector.memset(hist[:, :, :, :], 0.0)
            nc.vector.memset(histbf[:, :, :, :], 0.0)
            pjt = sg.tile([128, 4, BP, S_T], bf16)
            pji = sg.tile([128, 4, BP, S_I], bf16)
            Pt = sg.tile([128, BP, H], bf16)
            Pi0 = sg.tile([128, BP, H], bf16)
            Pi1 = sg.tile([128, BP, H], bf16)
            LTb = sg.tile([128, 4, MPAD], bf16)
            nc.vector.memset(LTb[:, :, :], 0.0)

            # =============== hoist phase ===============
            with tc.tile_pool(name="hoist_sb", bufs=1) as hsb, \
                 tc.tile_pool(name="hoist_ps", bufs=2, space="PSUM") as hps:
                ctb = hsb.tile([128, 4, BP, S_T], bf16)
                nc.sync.dma_start(
                    out=ctb,
                    in_=pk[:, poff["ctb"]:poff["ctb"] + 4 * BP * S_T]
                    .rearrange("p (k b s) -> p k b s", k=4, b=BP))
                cib = hsb.tile([128, 4, BP, S_I], bf16)
                nc.sync.dma_start(
                    out=cib,
                    in_=pk[:, poff["cib"]:poff["cib"] + 4 * BP * S_I]
                    .rearrange("p (k b s) -> p k b s", k=4, b=BP))
                yeb = hsb.tile([128, 4, nsteps * BP], bf16)
                nc.sync.dma_start(
                    out=yeb,
                    in_=pk[:, poff["yeb"]:poff["yeb"] + 4 * nsteps * BP]
                    .rearrange("p (k m) -> p k m", k=4))
                c2t = hsb.tile([128, 4, C], bf16)
                nc.sync.dma_start(out=c2t, in_=ld("c2t", 4))
                c2i = hsb.tile([128, 4, C], bf16)
                nc.sync.dma_start(out=c2i, in_=ld("c2i", 4))
                wft = hsb.tile([128, 4, H], bf16)
                nc.sync.dma_start(out=wft, in_=ld("wft", 4))
                wfi = hsb.tile([128, 4, H], bf16)
                nc.sync.dma_start(out=wfi, in_=ld("wfi", 4))
                w0i = hsb.tile([128, 4, 3 * H], bf16)
                nc.sync.dma_start(out=w0i, in_=ld("w0i", 4))

                # gi0 = yemb @ W0i.T + b0fold
                for mt in range(12):
                    p = hps.tile([128, nsteps * BP], f32, tag="gi0")
                    for kc in range(4):
                        nc.tensor.matmul(p[:, :], w0i[:, kc, mt * 128:(mt + 1) * 128],
                                         yeb[:, kc, :], start=(kc == 0), stop=(kc == 3))
                    nc.vector.tensor_scalar(
                        out=gi0[:, mt, :, :].rearrange("p t b -> p (t b)"),
                        in0=p[:, :],
                        scalar1=b0f[:, mt:mt + 1], scalar2=None, op0=OP.add)

                # projT txt: [128d, 400] per d-chunk
                for dk in range(4):
                    p = hps.tile([128, BP, S_T], f32, tag="pjt")
                    for kc in range(4):
                        nc.tensor.matmul(p[:, :, :], c2t[:, kc, dk * 128:(dk + 1) * 128],
                                         ctb[:, kc, :, :], start=(kc == 0), stop=(kc == 3))
                    nc.vector.tensor_copy(pjt[:, dk, :, :], p[:, :, :])
                # projT img: rhs split in b-pairs (N=392), one bank per pair
                for dk in range(4):
                    for q in range(4):
                        p = hps.tile([128, 2, S_I], f32, tag="pji")
                        for kc in range(4):
                            nc.tensor.matmul(p[:, :, :], c2i[:, kc, dk * 128:(dk + 1) * 128],
                                             cib[:, kc, 2 * q:2 * q + 2, :],
                                             start=(kc == 0), stop=(kc == 3))
                        nc.vector.tensor_copy(pji[:, dk, 2 * q:2 * q + 2, :], p[:, :, :])

                # P tiles (fusion pre-projection of ctx)
                for b in range(BP):
                    p = hps.tile([128, H], f32, tag="P")
                    for kc in range(4):
                        nc.tensor.matmul(p[:S_T, :], ctb[:, kc, b, :], wft[:, kc, :],
                                         start=(kc == 0), stop=(kc == 3))
                    nc.vector.tensor_copy(Pt[0:S_T, b, :], p[0:S_T, :])
                for b in range(BP):
                    p = hps.tile([128, H], f32, tag="P")
                    for kc in range(4):
                        nc.tensor.matmul(p[:, :], cib[:, kc, b, 0:128], wfi[:, kc, :],
                                         start=(kc == 0), stop=(kc == 3))
                    nc.vector.tensor_copy(Pi0[:, b, :], p[:, :])
                    p = hps.tile([128, H], f32, tag="P")
                    for kc in range(4):
                        nc.tensor.matmul(p[:S_I - 128, :], cib[:, kc, b, 128:S_I], wfi[:, kc, :],
                                         start=(kc == 0), stop=(kc == 3))
                    nc.vector.tensor_copy(Pi1[0:S_I - 128, b, :], p[0:S_I - 128, :])

            # =============== recurrence ===============
            with tc.tile_pool(name="ps_gate", bufs=1, space="PSUM") as psg, \
                 tc.tile_pool(name="ps_g1n", bufs=1, space="PSUM") as psn, \
                 tc.tile_pool(name="ps_ht", bufs=1, space="PSUM") as psh, \
                 tc.tile_pool(name="ps_sc", bufs=1, space="PSUM") as pssc, \
                 tc.tile_pool(name="ps_den", bufs=1, space="PSUM") as psd, \
                 tc.tile_pool(name="ps_fz", bufs=1, space="PSUM") as psf, \
                 tc.tile_pool(name="ps_tr", bufs=1, space="PSUM") as pst, \
                 tc.tile_pool(name="step", bufs=2) as st, \
                 tc.tile_pool(name="epool", bufs=1) as ep:

                for t in range(nsteps if phase >= 2 else 0):
                    # ---- GRU0: gh0.T = W0h stationary @ h.T
                    pg = psg.tile([128, 12, BP], f32, tag="g")
                    for kc in range(4):
                        for mt in range(12):
                            nc.tensor.matmul(pg[:, mt, :], w0h[:, kc, mt * 128:(mt + 1) * 128],
                                             histbf[:, kc, t, :],
                                             start=(kc == 0),
                                             stop=(kc == 3 and not use_bhh0))
                    if use_bhh0:
                        for mt in range(12):
                            nc.tensor.matmul(pg[:, mt, :], a0h[:, mt * 128:(mt + 1) * 128],
                                             ones_bf[0:1, 0:BP], start=False, stop=True)
                    Arz = st.tile([128, 8, BP], f32, tag="Arz")
                    nc.vector.tensor_tensor(
                        out=Arz[:, :, :], in0=gi0[:, 0:8, t, :],
                        in1=pg[:, 0:8, :], op=OP.add)
                    r0 = st.tile([128, 4, BP], f32, tag="r0")
                    # sigmoid(x) = 0.5 + 0.5*tanh(x/2): keeps the ACT engine on
                    # one table (Tanh) all step - saves ~3 table loads/step
                    nc.scalar.activation(r0[:, :, :], Arz[:, 0:4, :], AF.Tanh, scale=0.5)
                    nc.vector.tensor_scalar(out=r0[:, :, :], in0=r0[:, :, :],
                                            scalar1=0.5, scalar2=0.5,
                                            op0=OP.mult, op1=OP.add)
                    z0 = st.tile([128, 4, BP], f32, tag="z0")
                    nc.scalar.activation(z0[:, :, :], Arz[:, 4:8, :], AF.Tanh, scale=0.5)
                    nc.vector.tensor_scalar(out=z0[:, :, :], in0=z0[:, :, :],
                                            scalar1=0.5, scalar2=0.5,
                                            op0=OP.mult, op1=OP.add)
                    tn = st.tile([128, 4, BP], f32, tag="tn")
                    nc.vector.tensor_tensor(out=tn[:, :, :], in0=r0[:, :, :],
                                            in1=pg[:, 8:12, :], op=OP.mult)
                    nc.vector.tensor_tensor(out=tn[:, :, :], in0=tn[:, :, :],
                                            in1=gi0[:, 8:12, t, :], op=OP.add)
                    n0 = st.tile([128, 4, BP], f32, tag="n0")
                    nc.scalar.activation(n0[:, :, :], tn[:, :, :], AF.Tanh)
                    # h1 = n0 + z0*(h - n0)
                    d0 = st.tile([128, 4, BP], f32, tag="d0")
                    nc.vector.tensor_tensor(out=d0[:, :, :], in0=hist[:, :, t, :],
                                            in1=n0[:, :, :], op=OP.subtract)
                    nc.vector.tensor_tensor(out=d0[:, :, :], in0=z0[:, :, :],
                                            in1=d0[:, :, :], op=OP.mult)
                    h1 = st.tile([128, 4, BP], f32, tag="h1")
                    nc.vector.tensor_tensor(out=h1[:, :, :], in0=n0[:, :, :],
                                            in1=d0[:, :, :], op=OP.add)
                    h1b = st.tile([128, 4, BP], bf16, tag="h1b")
                    nc.vector.tensor_copy(h1b[:, :, :], h1[:, :, :])

                    # ---- hterm.T = hid2ctx stationary @ h1.T  (both modalities)
                    ph = psh.tile([128, 2, 4, BP], f32, tag="ht")
                    for kc in range(4):
                        for dk in range(4):
                            nc.tensor.matmul(ph[:, 0, dk, :], hdt[:, kc, dk * 128:(dk + 1) * 128],
                                             h1b[:, kc, :], start=(kc == 0), stop=(kc == 3))
                            nc.tensor.matmul(ph[:, 1, dk, :], hdi[:, kc, dk * 128:(dk + 1) * 128],
                                             h1b[:, kc, :], start=(kc == 0), stop=(kc == 3))
                    htT = st.tile([128, 2, 4, BP], f32, tag="htT")
                    nc.vector.tensor_copy(
                        htT[:, :, :, :].rearrange("p m k b -> p (m k b)"),
                        ph[:, :, :, :].rearrange("p m k b -> p (m k b)"))

                    # ---- e = tanh(proj + hterm): broadcast DVE add (hterm dup
                    # along s) + wide bias-free ACT tanh, instead of 64 small
                    # per-(dk,b) biased activations
                    ept = ep.tile([128, 4, BP, S_T], f32, tag="ept")
                    epi = ep.tile([128, 4, BP, S_I], f32, tag="epi")
                    for dk in range(4):
                        nc.vector.tensor_tensor(
                            out=ept[:, dk, :, :], in0=pjt[:, dk, :, :],
                            in1=htT[:, 0, dk, :, None].broadcast_to((128, BP, S_T)),
                            op=OP.add)
                        nc.vector.tensor_tensor(
                            out=epi[:, dk, :, :], in0=pji[:, dk, :, :],
                            in1=htT[:, 1, dk, :, None].broadcast_to((128, BP, S_I)),
                            op=OP.add)
                    eTt = ep.tile([128, 4, BP, S_T], bf16, tag="eTt")
                    eTi = ep.tile([128, 4, BP, S_I], bf16, tag="eTi")
                    nc.scalar.activation(
                        eTt[:, :, :, :].rearrange("p k b s -> p (k b s)"),
                        ept[:, :, :, :].rearrange("p k b s -> p (k b s)"), AF.Tanh)
                    nc.scalar.activation(
                        eTi[:, :, :, :].rearrange("p k b s -> p (k b s)"),
                        epi[:, :, :, :].rearrange("p k b s -> p (k b s)"), AF.Tanh)

                    # ---- scores.T [s, b] = e.T stationary @ v
                    psc = pssc.tile([128, 3, BP], f32, tag="sc")
                    for b in range(BP):
                        for dk in range(4):
                            nc.tensor.matmul(psc[0:S_T, 0, b:b + 1], eTt[:, dk, b, :],
                                             vtt[:, dk, :], start=(dk == 0), stop=(dk == 3))
                            nc.tensor.matmul(psc[0:128, 1, b:b + 1], eTi[:, dk, b, 0:128],
                                             vit[:, dk, :], start=(dk == 0), stop=(dk == 3))
                            nc.tensor.matmul(psc[0:S_I - 128, 2, b:b + 1], eTi[:, dk, b, 128:S_I],
                                             vit[:, dk, :], start=(dk == 0), stop=(dk == 3))
                    if use_mask:
                        nc.vector.tensor_tensor(out=psc[0:S_T, 0, :], in0=psc[0:S_T, 0, :],
                                                in1=maddt[0:S_T, :], op=OP.add)
                    # ---- w = exp(scores)
                    wTt = st.tile([128, BP], bf16, tag="wTt")
                    wTi0 = st.tile([128, BP], bf16, tag="wTi0")
                    wTi1 = st.tile([128, BP], bf16, tag="wTi1")
                    nc.scalar.activation(wTt[0:S_T, :], psc[0:S_T, 0, :], AF.Exp)
                    nc.scalar.activation(wTi0[:, :], psc[:, 1, :], AF.Exp)
                    nc.scalar.activation(wTi1[0:S_I - 128, :], psc[0:S_I - 128, 2, :], AF.Exp)

                    # ---- denominators + reciprocal scatter
                    pd = psd.tile([128, 8], f32, tag="den")
                    nc.tensor.matmul(pd[0:8, 0:1], wTt[0:S_T, :], ones_bf[0:S_T, 0:1],
                                     start=True, stop=True)
                    nc.tensor.matmul(pd[0:8, 1:2], wTi0[:, :], ones_bf[:, 0:1],
                                     start=True, stop=False)
                    nc.tensor.matmul(pd[0:8, 1:2], wTi1[0:S_I - 128, :], ones_bf[0:S_I - 128, 0:1],
                                     start=False, stop=True)
                    rdf = st.tile([8, 2], f32, tag="rdf")
                    nc.vector.reciprocal(rdf[:, :], pd[0:8, 0:2])
                    rdb = st.tile([8, 2], bf16, tag="rdb")
                    nc.vector.tensor_copy(rdb[:, :], rdf[:, :])
                    for g in range(2):
                        nc.tensor.matmul(pd[:, 2 + 2 * g:4 + 2 * g], ind[:, g, :], rdb[:, :],
                                         start=True, stop=True)
                    rds = st.tile([128, 2, 2], f32, tag="rds")
                    nc.vector.tensor_copy(rds[:, :, :].rearrange("p g x -> p (g x)"),
                                          pd[:, 2:6])

                    # ---- weighted sums of P (fusion input), col-packed 4 b/bank
                    fzpre = st.tile([128, 2, H], f32, tag="fzpre")
                    tmpc = st.tile([128, H], f32, tag="tmpc")
                    for g in range(2):
                        pa = psf.tile([128, H], f32, tag="fzA")
                        pb = psf.tile([128, H], f32, tag="fzB")
                        for j in range(4):
                            b = 4 * g + j
                            nc.tensor.matmul(pa[32 * j:32 * j + 1, :], wTt[0:S_T, b:b + 1],
                                             Pt[0:S_T, b, :], start=True, stop=True,
                                             tile_position=(0, 32 * j))
                            nc.tensor.matmul(pb[32 * j:32 * j + 1, :], wTi0[:, b:b + 1],
                                             Pi0[:, b, :], start=True, stop=False,
                                             tile_position=(0, 32 * j))
                            nc.tensor.matmul(pb[32 * j:32 * j + 1, :], wTi1[0:S_I - 128, b:b + 1],
                                             Pi1[0:S_I - 128, b, :], start=False, stop=True,
                                             tile_position=(0, 32 * j))
                        nc.vector.tensor_scalar(out=tmpc[:, :], in0=pb[:, :],
                                                scalar1=rds[:, g, 1:2], scalar2=None,
                                                op0=OP.mult)
                        nc.vector.scalar_tensor_tensor(
                            out=fzpre[:, g, :], in0=pa[:, :], scalar=rds[:, g, 0:1],
                            in1=tmpc[:, :], op0=OP.mult, op1=OP.add)
                    fzf = st.tile([128, 2, H], f32, tag="fzf")
                    nc.scalar.activation(fzf[:, :, :], fzpre[:, :, :], AF.Tanh)

                    # ---- transpose fz [8b, 512] -> fzT [128c, 4kc, 8b]
                    fzT = st.tile([128, 4, BP], bf16, tag="fzT")
                    for g in range(2):
                        for ck in range(4):
                            ptr = pst.tile([128, 128], f32, tag="tr")
                            nc.tensor.transpose(ptr[:, :], fzf[:, g, ck * 128:(ck + 1) * 128],
                                                ident[:, :])
                            nc.vector.tensor_copy(fzT[:, ck, 4 * g:4 * g + 4],
                                                  ptr[:, 0:128:32])

                    # ---- GRU1
                    pg1 = psg.tile([128, 12, BP], f32, tag="g")
                    pn1 = psn.tile([128, 4, BP], f32, tag="gn")
                    for kc in range(4):
                        for mt in range(12):
                            nc.tensor.matmul(pg1[:, mt, :], w1i[:, kc, mt * 128:(mt + 1) * 128],
                                             fzT[:, kc, :], start=(kc == 0),
                                             stop=(kc == 3 and mt >= 8 and not use_b1i))
                    if use_b1i:
                        for mt in range(12):
                            nc.tensor.matmul(pg1[:, mt, :], a1i[:, mt * 128:(mt + 1) * 128],
                                             ones_bf[0:1, 0:BP], start=False, stop=(mt >= 8))
                    for kc in range(4):
                        for mt in range(8):
                            nc.tensor.matmul(pg1[:, mt, :], w1h[:, kc, mt * 128:(mt + 1) * 128],
                                             h1b[:, kc, :], start=False,
                                             stop=(kc == 3 and not use_b1h))
                        for mt in range(4):
                            nc.tensor.matmul(pn1[:, mt, :], w1h[:, kc, (8 + mt) * 128:(9 + mt) * 128],
                                             h1b[:, kc, :], start=(kc == 0),
                                             stop=(kc == 3 and not use_b1h))
                    if use_b1h:
                        for mt in range(8):
                            nc.tensor.matmul(pg1[:, mt, :], a1h[:, mt * 128:(mt + 1) * 128],
                                             ones_bf[0:1, 0:BP], start=False, stop=True)
                        for mt in range(4):
                            nc.tensor.matmul(pn1[:, mt, :], a1h[:, (8 + mt) * 128:(9 + mt) * 128],
                                             ones_bf[0:1, 0:BP], start=False, stop=True)
                    r1 = st.tile([128, 4, BP], f32, tag="r0")
                    nc.scalar.activation(r1[:, :, :], pg1[:, 0:4, :], AF.Tanh, scale=0.5)
                    nc.vector.tensor_scalar(out=r1[:, :, :], in0=r1[:, :, :],
                                            scalar1=0.5, scalar2=0.5,
                                            op0=OP.mult, op1=OP.add)
                    z1 = st.tile([128, 4, BP], f32, tag="z0")
                    nc.scalar.activation(z1[:, :, :], pg1[:, 4:8, :], AF.Tanh, scale=0.5)
                    nc.vector.tensor_scalar(out=z1[:, :, :], in0=z1[:, :, :],
                                            scalar1=0.5, scalar2=0.5,
                                            op0=OP.mult, op1=OP.add)
                    tn1 = st.tile([128, 4, BP], f32, tag="tn")
                    nc.vector.tensor_tensor(out=tn1[:, :, :], in0=r1[:, :, :],
                                            in1=pn1[:, :, :], op=OP.mult)
                    nc.vector.tensor_tensor(out=tn1[:, :, :], in0=tn1[:, :, :],
                                            in1=pg1[:, 8:12, :], op=OP.add)
                    n1 = st.tile([128, 4, BP], f32, tag="n0")
                    nc.scalar.activation(n1[:, :, :], tn1[:, :, :], AF.Tanh)
                    d1 = st.tile([128, 4, BP], f32, tag="d0")
                    nc.vector.tensor_tensor(out=d1[:, :, :], in0=h1[:, :, :],
                                            in1=n1[:, :, :], op=OP.subtract)
                    nc.vector.tensor_tensor(out=d1[:, :, :], in0=z1[:, :, :],
                                            in1=d1[:, :, :], op=OP.mult)
                    nc.vector.tensor_tensor(out=hist[:, :, t + 1, :], in0=n1[:, :, :],
                                            in1=d1[:, :, :], op=OP.add)
                    nc.vector.tensor_copy(histbf[:, :, t + 1, :], hist[:, :, t + 1, :])

            # =============== logits + vocab phase ===============
            m_rows = nsteps * BP
            nmt = (m_rows + 127) // 128
            with tc.tile_pool(name="ps_L", bufs=2, space="PSUM") as psL:
              if phase >= 3:
                  for e in range(4):
                      p = psL.tile([128, m_rows], f32, tag="L")
                      for kc in range(4):
                          nc.tensor.matmul(
                              p[:, :], h2o[:, kc, e * 128:(e + 1) * 128],
                              histbf[:, kc, 1:nsteps + 1, :].rearrange("p t b -> p (t b)"),
                              start=(kc == 0), stop=(kc == 3))
                      nc.scalar.activation(LTb[:, e, 0:m_rows], p[:, :], AF.Tanh,
                                           bias=h2obt[:, e:e + 1])

            with tc.tile_pool(name="wv", bufs=3) as wvp, \
                 tc.tile_pool(name="ps_z", bufs=4, space="PSUM") as psz, \
                 tc.tile_pool(name="ps_zt", bufs=1, space="PSUM") as pszt, \
                 tc.tile_pool(name="vocab_sb", bufs=3) as vsb:
              if phase >= 4:
                  Sacc = sg.tile([128, 3, NVS], f32)
                  Srow = sg.tile([128, 4], f32)
                  for vs in range(NVS):
                      n = min(VS, V - vs * VS)
                      wvt = wvp.tile([128, 4, VS], bf16, tag="wv")
                      nc.sync.dma_start(
                          out=wvt[:, :, 0:n],
                          in_=WvT[:, vs * VS:vs * VS + n].rearrange("(k p) v -> p k v", p=128))
                      for mt in range(nmt):
                          pz = psz.tile([128, VS], f32, tag="z")
                          for e in range(4):
                              nc.tensor.matmul(pz[:, 0:n], LTb[:, e, mt * 128:(mt + 1) * 128],
                                               wvt[:, e, 0:n], start=(e == 0),
                                               stop=(e == 3 and not use_bv))
                          if use_bv:
                              nc.tensor.matmul(pz[:, 0:n], ones_row[0:1, :],
                                               bvt[:, vs * VS:vs * VS + n], start=False, stop=True)
                          scr = vsb.tile([128, VS], bf16, tag="scr")
                          nc.scalar.activation(scr[:, 0:n], pz[:, 0:n], AF.Exp,
                                               accum_out=Sacc[:, mt, vs:vs + 1])
                  for mt in range(nmt):
                      nc.vector.reduce_sum(Srow[:, mt:mt + 1], Sacc[:, mt, :],
                                           axis=mybir.AxisListType.X)

                  # ---- z_target on device: ztot = sum_m L[m,:].Wv[:,y_next[m]]
                  ztmp = sg.tile([128, 4, M], f32)
                  nc.vector.tensor_tensor(out=ztmp[:, :, :], in0=LTb[:, :, 0:M],
                                          in1=wtg[:, :, :], op=OP.mult)
                  ztp = sg.tile([128, 1], f32)
                  nc.vector.reduce_sum(ztp[:, 0:1],
                                       ztmp[:, :, :].rearrange("p k m -> p (k m)"),
                                       axis=mybir.AxisListType.X)
                  pzt = pszt.tile([128, 1], f32, tag="zt")
                  nc.tensor.matmul(pzt[0:1, 0:1], ztp[:, 0:1], ones_f[:, 0:1],
                                   start=True, stop=True)
                  nc.vector.tensor_copy(Srow[0:1, 3:4], pzt[0:1, 0:1])
                  nc.sync.dma_start(out=S_out[:, :], in_=Srow[:, :])

    _split_waits(nc)
    return nc


def _to_bf(x):
    return np.asarray(x, dtype=np.float32).astype(ml_dtypes.bfloat16)


def _r4flat(a):
    """[512, D] -> [128, 4*D] with block[p, k*D+d] = a[k*128+p, d]."""
    a = np.asarray(a)
    D = a.shape[1]
    return np.ascontiguousarray(a.reshape(4, 128, D).transpose(1, 0, 2).reshape(128, 4 * D))


def kernel(**inputs):
    txt_ctx = np.asarray(inputs["txt_ctx"], np.float32)
    txt_mask = np.asarray(inputs["txt_mask"], np.float32)
    img_ctx = np.asarray(inputs["img_ctx"], np.float32)
    y = np.asarray(inputs["y"])
    emb_W = np.asarray(inputs["emb_W"], np.float32)
    d0Wih = np.asarray(inputs["dec0_Wih"], np.float32)
    d0Whh = np.asarray(inputs["dec0_Whh"], np.float32)
    d0bih = np.asarray(inputs["dec0_bih"], np.float32)
    d0bhh = np.asarray(inputs["dec0_bhh"], np.float32)
    d1Wih = np.asarray(inputs["dec1_Wih"], np.float32)
    d1Whh = np.asarray(inputs["dec1_Whh"], np.float32)
    d1bih = np.asarray(inputs["dec1_bih"], np.float32)
    d1bhh = np.asarray(inputs["dec1_bhh"], np.float32)
    t_c2c = np.asarray(inputs["txt_ctx2ctx"], np.float32)
    t_h2c = np.asarray(inputs["txt_hid2ctx"], np.float32)
    t_v = np.asarray(inputs["txt_mlp_v"], np.float32)
    i_c2c = np.asarray(inputs["img_ctx2ctx"], np.float32)
    i_h2c = np.asarray(inputs["img_hid2ctx"], np.float32)
    i_v = np.asarray(inputs["img_mlp_v"], np.float32)
    fusion_W = np.asarray(inputs["fusion_W"], np.float32)
    h2oW = np.asarray(inputs["hid2out_W"], np.float32)
    h2ob_v = np.asarray(inputs["hid2out_b"], np.float32)
    o2pW = np.asarray(inputs["out2prob_W"], np.float32)
    o2pb = np.asarray(inputs["out2prob_b"], np.float32)

    use_mask = not np.all(txt_mask > 0)
    use_bhh0 = bool(np.any(d0bhh != 0))
    use_b1i = bool(np.any(d1bih != 0))
    use_b1h = bool(np.any(d1bhh != 0))
    use_bv = bool(np.any(o2pb != 0))

    nsteps = NSTEPS
    m_rows = nsteps * BP
    key = ("nc", nsteps, use_mask, use_bhh0, use_b1i, use_b1h, use_bv)
    if key not in _cache:
        _cache[key] = build_nc(nsteps, use_mask, use_bhh0, use_b1i, use_b1h, use_bv)
    nc = _cache[key]

    poff, NPK = _pack_layout(use_bhh0, use_b1i, use_b1h)
    foff, NPKF = _packf_layout(use_mask)

    embz = emb_W.copy()
    embz[0, :] = 0.0

    WvT_b = _to_bf(o2pW.T)                       # [E, V] bf16

    # shared (batch-independent) bf16 pack blocks
    pk_shared = np.zeros((128, NPK), ml_dtypes.bfloat16)

    def put(name, block):
        sz = dict(PACK_BASE).get(name)
        if sz is None:
            sz = 3 * H
        assert block.shape == (128, sz) or block.shape[1] == sz, (name, block.shape)
        pk_shared[:, poff[name]:poff[name] + block.shape[1]] = block

    put("w0h", _r4flat(_to_bf(d0Whh.T)))
    put("w1i", _r4flat(_to_bf(d1Wih.T)))
    put("w1h", _r4flat(_to_bf(d1Whh.T)))
    put("hdt", _r4flat(_to_bf(t_h2c.T)))
    put("hdi", _r4flat(_to_bf(i_h2c.T)))
    put("h2o", _r4flat(_to_bf(h2oW.T)))
    put("vtt", _r4flat(_to_bf(t_v[:, None])))
    put("vit", _r4flat(_to_bf(i_v[:, None])))
    put("c2t", _r4flat(_to_bf(t_c2c.T)))
    put("c2i", _r4flat(_to_bf(i_c2c.T)))
    put("wft", _r4flat(_to_bf(fusion_W[:, 0:C].T)))
    put("wfi", _r4flat(_to_bf(fusion_W[:, C:2 * C].T)))
    put("w0i", _r4flat(_to_bf(d0Wih.T)))
    ind = np.zeros((2, 8, 128), np.float32)
    for b in range(8):
        ind[b // 4, b, 32 * (b % 4)] = 1.0
    indblk = np.zeros((128, 256), ml_dtypes.bfloat16)
    indblk[0:8, :] = _to_bf(ind).transpose(1, 0, 2).reshape(8, 256)
    put("ind", indblk)
    if use_bhh0:
        blk = np.zeros((128, 3 * H), ml_dtypes.bfloat16)
        blk[0, :] = _to_bf(d0bhh)
        pk_shared[:, poff["a0h"]:poff["a0h"] + 3 * H] = blk
    if use_b1i:
        blk = np.zeros((128, 3 * H), ml_dtypes.bfloat16)
        blk[0, :] = _to_bf(d1bih)
        pk_shared[:, poff["a1i"]:poff["a1i"] + 3 * H] = blk
    if use_b1h:
        blk = np.zeros((128, 3 * H), ml_dtypes.bfloat16)
        blk[0, :] = _to_bf(d1bhh)
        pk_shared[:, poff["a1h"]:poff["a1h"] + 3 * H] = blk

    # shared f32 pack
    b0fold_v = d0bih.copy()
    b0fold_v[0:2 * H] += d0bhh[0:2 * H]
    pkf_shared = np.zeros((128, NPKF), np.float32)
    pkf_shared[:, foff["h2ob"]:foff["h2ob"] + 4] = h2ob_v.reshape(4, 128).T
    pkf_shared[:, foff["b0f"]:foff["b0f"] + 12] = b0fold_v.reshape(12, 128).T

    in_maps = []
    for c in range(NCORES):
        bs = slice(c * BP, (c + 1) * BP)
        y_c = y[:, bs].astype(np.int64)
        yemb = embz[y_c[0:nsteps].reshape(-1)].reshape(nsteps, BP, E)
        y_next = y_c[1:nsteps + 1].reshape(-1)          # [M] targets, m = t*BP+b
        pk = pk_shared.copy()
        # ctxT blocks: [512(c), BP, S] -> r4flat over (b,s)
        ct = _to_bf(txt_ctx[:, bs, :].transpose(2, 1, 0))      # [C, BP, S_T]
        pk[:, poff["ctb"]:poff["ctb"] + 4 * BP * S_T] = \
            _r4flat(ct.reshape(C, BP * S_T))
        ci = _to_bf(img_ctx[:, bs, :].transpose(2, 1, 0))      # [C, BP, S_I]
        pk[:, poff["cib"]:poff["cib"] + 4 * BP * S_I] = \
            _r4flat(ci.reshape(C, BP * S_I))
        ye = _to_bf(yemb.transpose(2, 0, 1))                   # [E, nsteps, BP]
        pk[:, poff["yeb"]:poff["yeb"] + 4 * nsteps * BP] = \
            _r4flat(ye.reshape(E, nsteps * BP))
        # wtg: WvT columns at targets, but m-index must match LTb's m = t*BP+b
        wt = WvT_b[:, y_next]                                  # [E, M] bf16
        pk[:, poff["wtg"]:poff["wtg"] + 4 * M] = _r4flat(wt)
        m = {"pk": pk, "pkf": pkf_shared, "WvT": WvT_b}
        if use_mask:
            pf = pkf_shared.copy()
            madd = np.zeros((128, BP), np.float32)
            madd[0:S_T, :] = np.where(txt_mask[:, bs] > 0, 0.0, NEG)
            pf[:, foff["madd"]:foff["madd"] + BP] = madd
            m["pkf"] = pf
        if use_bv:
            m["bvT"] = _to_bf(o2pb[None, :])
        in_maps.append(m)

    global LAST_NC, LAST_IN_MAPS
    LAST_NC, LAST_IN_MAPS = nc, in_maps
    res = run_bass_kernel_spmd(nc, in_maps, core_ids=list(range(NCORES)))

    # host reduction: loss = sum log(S_row) - sum z_target
    total = np.float64(0.0)
    for c in range(NCORES):
        r = res.results[c]
        S_flat = r["S"][:, 0:3].T.reshape(-1)[:m_rows]     # row-major m = mt*128+p
        ztot = np.float64(r["S"][0, 3])
        total += np.log(S_flat.astype(np.float64)).sum() - ztot
        if use_bv:
            # device ztot covers L@Wv[:,y]; the out2prob bias at the target
            # (included in the device logits for S) is added here
            y_next_c = y[1:nsteps + 1, c * BP:(c + 1) * BP].astype(np.int64).reshape(-1)
            total -= np.float64(o2pb[y_next_c].astype(np.float64).sum())
    return np.float32(total)


if __name__ == "__main__":
    pass
